# revision 12
# baseline (speedup 1.0000x reference)
"""AttentionalPropagation (SuperGlue-style GNN message passing) on 8 trn2 cores.

Problem (hardcoded): B=2, D=256, N=M=4096, H=4 heads, head dim 64.
  q = P_q(x); k = P_k(source); v = P_v(source)      (bottleneck 1x1 convs D->D/8->D)
  msg = attn(q, k, v); merged = P_m(msg)            (per-head softmax over M)
  out = Conv(relu(BN(Conv(cat[x, merged]))))        (512->64->256)

Sharding: 8 cores = (batch b in {0,1}) x (query chunk of 1024).  Each core
computes k1/v1 for its full batch row and attention + MLP for its 1024 query
columns.  Weights replicated.  No collectives.

Algebraic folds (host side):
  * scores = k1e^T C_h q1e with C_h = Ak'_h @ Aq'_h^T (33x33), where Ak'/Aq'
    are the bias-extended per-head second-projection blocks with the
    first-projection biases folded into their ones-rows.  Neither Wk2 nor Wq2
    ever runs on device.
  * Wv2 never runs on device either: msg_h = Wv2_h (sum_m p_m v1e_m)/denom,
    and Wv2/Wm1/Wm2 + all biases collapse into wmm_h = Wm2 @ [bias_h | W_h]
    applied directly to the normalized (33-row) v1-space message.
  * BN scale folds into Wp1, BN shift + conv bias into the relu bias.

Attention layout: scores computed transposed (keys m on partitions, queries n
free).  kv1_all rows: 0-31 k1(raw), 32-63 v1(raw), 64 ones, 65-127 zero.
v1e^T (per 128-m-chunk, [ones|v1] columns) is produced by a tiny selector
matmul; the msg matmul contracts it against exp(scores) in fp8 with
perf_mode=DoubleRow (virtual K=256), row 0 of the msg PSUM = softmax denom.

exp runs split across two engines: ACT (exact, fp8 out) and DVE (Schraudolph
bit-trick: bits8 = round(s/ln2 + 56) written as int8, bitcast to fp8e4).

HAM note: all hot matmuls are full-K (128 partitions); the DR msg matmuls are
128-partition too.  Small-K matmuls (merge/mlp/q) ride the warm clock.
"""

import numpy as np

import concourse.bass as bass
import concourse.mybir as mybir
import concourse.tile as tile
from concourse import bacc, bass_utils

B, D, N, M, H = 2, 256, 4096, 4096, 4
DIM = D // H       # 64
D8 = D // 8        # 32
TD = 2 * D         # 512
TD8 = TD // 8      # 64
BN_EPS = 1e-5
NCORES = 8
NCHUNK = N // 4    # query columns per core
NT = 512           # n tile (PSUM bank = 512 fp32)
NTILES = NCHUNK // NT          # 2
MT = 512           # source m tile for k/v projection stage
MTILES = M // MT               # 8
MC = 128           # m chunk (scores PSUM partition dim)
MCHUNKS = M // MC              # 32
NSTEP = MCHUNKS // 2           # 16 steps (2 chunks each) per (nt, h)
F32 = mybir.dt.float32
F32R = mybir.dt.float32r
BF16 = mybir.dt.bfloat16
FP8 = mybir.dt.float8e4
I8 = mybir.dt.int8
AF = mybir.ActivationFunctionType
ALU = mybir.AluOpType

WARMUP_MMS = 20
# Schraudolph constants for fp8e4(m3) bits of exp(0.125*s):
# bits = s*(0.125*8/ln2) + 8*7 - 0.458 + 0.5(trunc comp)
EXP_A = 0.125 * 8.0 / np.log(2.0)
EXP_B = 56.0 - 0.458 + 0.5

# ---- bf16 const pack offsets (partition-padded to 128) ----
CB_WK = 0            # [128, 2, 32]
CB_WV = 64           # [128, 2, 32]
CB_CHT = 128         # [33, H, 128]
CB_IV = 640          # [65, 34]
CB_WMM = 674         # [33, H, 2, 128]
CB_WP1M = 1698       # [128, 2, 64]
CB_END = 1826
# ---- f32 const pack offsets ----
CF_WQ = 0            # [128, 2, 32]
CF_WP1X = 64         # [128, 2, 64]
CF_WP2 = 192         # [65, 2, 128]
CF_BE1 = 448         # [64, 1]
CF_END = 449


def dve_step(nt, h, bi):
    """Which exp steps run on DVE (Schraudolph) instead of ACT."""
    if h == 0:
        return bi in (3, 5, 7, 9, 11, 13)
    return bi in (2, 4, 6, 8, 10, 12, 14)


def build_body(ctx, tc: tile.TileContext, io):
    nc = tc.nc
    x_d = io["x_chunk"]          # [2, 128, NCHUNK]  (channel-chunk, partition, n)
    src_d = io["source_b"]       # [2, 128, M]
    out_d = io["out_chunk"]      # [2, 128, NCHUNK]

    consts = ctx.enter_context(tc.tile_pool(name="consts", bufs=1))
    big = ctx.enter_context(tc.tile_pool(name="big", bufs=1))
    srcp = ctx.enter_context(tc.tile_pool(name="srcp", bufs=3))
    ep = ctx.enter_context(tc.tile_pool(name="ep", bufs=4))
    nrm = ctx.enter_context(tc.tile_pool(name="nrm", bufs=4))

    # ---- const DMAs first: gpsimd FIFO must not stall them ----
    cb = consts.tile([128, CB_END], BF16)
    nc.gpsimd.dma_start(out=cb, in_=io["cb"])
    cf = consts.tile([128, CF_END], F32R)
    nc.gpsimd.dma_start(out=cf, in_=io["cf"])

    wk1 = lambda ct: cb[:, CB_WK + ct * D8: CB_WK + (ct + 1) * D8]
    wv1 = lambda ct: cb[:, CB_WV + ct * D8: CB_WV + (ct + 1) * D8]
    cht = lambda h: cb[0:33, CB_CHT + h * 128: CB_CHT + (h + 1) * 128]
    iv_ap = cb[0:65, CB_IV: CB_IV + 34]
    wmm = lambda h, ct: cb[0:33, CB_WMM + (h * 2 + ct) * 128: CB_WMM + (h * 2 + ct + 1) * 128]
    wp1m = lambda ct: cb[:, CB_WP1M + ct * TD8: CB_WP1M + (ct + 1) * TD8]
    wq1 = lambda ct: cf[:, CF_WQ + ct * D8: CF_WQ + (ct + 1) * D8]
    wp1x = lambda ct: cf[:, CF_WP1X + ct * TD8: CF_WP1X + (ct + 1) * TD8]
    wp2 = lambda ct: cf[0:TD8 + 1, CF_WP2 + ct * 128: CF_WP2 + (ct + 1) * 128]
    be1_ap = cf[0:TD8, CF_BE1: CF_BE1 + 1].bitcast(F32)

    # ---- persistent activations ----
    kv1_all = big.tile([128, M], BF16)   # rows 0-31 k1, 32-63 v1, 64 ones, 65+ 0
    v1t = big.tile([128, MCHUNKS, 48], FP8)   # [ones|v1e]^T per m chunk, 34 used
    x_sb = big.tile([128, 2, NCHUNK], F32R)
    for _ct in range(2):
        nc.sync.dma_start(out=x_sb[:, _ct, :], in_=x_d[_ct])
    qh_sb = big.tile([128, H, NCHUNK], BF16)          # C_h q1e, rows 33+ zero
    q1 = big.tile([33, NCHUNK], BF16)                 # rows 0-31 q1, 32 ones
    msg_sb = big.tile([33, H, NCHUNK], BF16)          # row 0 = 1, 1-32 mv1n
    mm_sb = big.tile([128, 2, NCHUNK], BF16)          # merged msg (mlp input)
    h1 = big.tile([TD8 + 1, NCHUNK], F32R)            # relu(BN(.)), row 64 ones
    out_sb = big.tile([128, 2, NCHUNK], F32)

    # ---- DVE memsets (gpsimd stays free for DMA descriptor gen) ----
    wza = consts.tile([128, 128], BF16)
    wzb = consts.tile([128, NT], BF16)
    nc.vector.memset(wza, 0.0)
    nc.vector.memset(wzb, 0.0)
    nc.vector.memset(kv1_all[64:128, :], 0.0)
    nc.gpsimd.memset(kv1_all[64:65, :], 1.0)
    nc.gpsimd.memset(q1[32:33, :], 1.0)
    nc.gpsimd.memset(h1[TD8:TD8 + 1, :].bitcast(F32), 1.0)

    # ---- PE warm-up while input DMAs stream in ----
    ppw = tc.tile_pool(name="ppw", bufs=2, space="PSUM")
    ppw_pool = ppw.__enter__()
    for i in range(WARMUP_MMS):
        pw = ppw_pool.tile([128, NT], F32, tag="pw", name="pw")
        nc.tensor.matmul(pw, wza, wzb, start=True, stop=True)
    ppw.__exit__(None, None, None)

    # ---- stage B: k1/v1 over full M + v1e transpose, SW-pipelined ----
    ppb = tc.tile_pool(name="ppb", bufs=3, space="PSUM")
    ppb_pool = ppb.__enter__()

    def emit_kv1(mt):
        ms = mt * MT
        src = srcp.tile([128, 2, MT], BF16, tag="src", name="src")
        for ct in range(2):
            nc.sync.dma_start(out=src[:, ct, :], in_=src_d[ct, :, ms:ms + MT])
        ps1 = ppb_pool.tile([64, MT], F32, tag="ps1", name="ps1")
        nc.tensor.matmul(ps1[0:D8, :], wk1(0), src[:, 0, :], start=True, stop=False)
        nc.tensor.matmul(ps1[0:D8, :], wk1(1), src[:, 1, :], start=False, stop=True)
        nc.tensor.matmul(ps1[D8:2 * D8, :], wv1(0), src[:, 0, :], start=True, stop=False)
        nc.tensor.matmul(ps1[D8:2 * D8, :], wv1(1), src[:, 1, :], start=False, stop=True)
        nc.vector.tensor_copy(out=kv1_all[0:64, ms:ms + MT], in_=ps1)

    def emit_v1t(mt):
        ms4 = mt * 4
        psv = ppb_pool.tile([128, 4, 34], F32, tag="psv", name="psv")
        for j in range(4):
            mc = ms4 + j
            nc.tensor.matmul(psv[:, j, :], kv1_all[0:65, mc * MC:(mc + 1) * MC],
                             iv_ap, start=True, stop=True)
        nc.vector.tensor_copy(out=v1t[:, ms4:ms4 + 4, 0:34], in_=psv)

    def emit_q1(nt):
        ns = nt * NT
        psq = ppb_pool.tile([D8, NT], F32, tag="ps1", name="psq")
        nc.tensor.matmul(psq, wq1(0), x_sb[:, 0, ns:ns + NT], start=True, stop=False)
        nc.tensor.matmul(psq, wq1(1), x_sb[:, 1, ns:ns + NT], start=False, stop=True)
        nc.vector.tensor_copy(out=q1[0:D8, ns:ns + NT], in_=psq)

    def emit_qh(h, nt):
        ns = nt * NT
        psq2 = ppb_pool.tile([128, NT], F32, tag="psv", name="psq2")
        nc.tensor.matmul(psq2, cht(h), q1[:, ns:ns + NT], start=True, stop=True)
        nc.scalar.copy(out=qh_sb[:, h, ns:ns + NT], in_=psq2)

    # q work woven into the kv1 stream so the full-K kv1 matmuls keep the
    # HAM clock warm through the whole preamble.
    emit_kv1(0)
    emit_kv1(1)
    emit_q1(0)
    emit_q1(1)
    for mt in range(2, MTILES):
        emit_kv1(mt)
        emit_v1t(mt - 2)
        if 3 <= mt <= 6:
            emit_qh(mt - 3, 0)
            emit_qh(mt - 3, 1)
    emit_v1t(MTILES - 2)
    emit_v1t(MTILES - 1)

    ppb.__exit__(None, None, None)

    # ---- attention: flat pipeline over (nt, h, bi), lookahead 2 ----
    pps = ctx.enter_context(tc.tile_pool(name="pps", bufs=3, space="PSUM"))
    ppm = ctx.enter_context(tc.tile_pool(name="ppm", bufs=2, space="PSUM"))

    def emit_scores(nt, h, bi):
        ns = nt * NT
        ps = pps.tile([128, 2, NT], F32, tag="ps", name="ps")
        for j in range(2):
            mc = bi * 2 + j
            nc.tensor.matmul(ps[:, j, :], kv1_all[:, mc * MC:(mc + 1) * MC],
                             qh_sb[:, h, ns:ns + NT], start=True, stop=True)
        e = ep.tile([128, 2, NT], FP8, tag="e", name="e")
        if dve_step(nt, h, bi):
            nc.vector.tensor_scalar(
                out=e[:, :, :].bitcast(I8), in0=ps, scalar1=float(EXP_A),
                scalar2=float(EXP_B), op0=ALU.mult, op1=ALU.add)
        else:
            nc.scalar.activation(out=e, in_=ps, func=AF.Exp, scale=0.125)
        return e

    def emit_norm(pm, h, ns):
        rec = nrm.tile([1, NT], F32, tag="rec", name="rec")
        nc.vector.reciprocal_approx_fast(out=rec, in_=pm[0:1, :])
        bc = nrm.tile([33, NT], F32, tag="bc", name="bc")
        nc.gpsimd.partition_broadcast(bc, rec)
        nc.vector.tensor_mul(out=msg_sb[0:33, h, ns:ns + NT],
                             in0=pm[0:33, :], in1=bc)

    def emit_warm_mms(n):
        # dummy full-K matmuls: hold the HAM clock at 8/8 through thin spots
        pw = ppm.tile([128, NT], F32, tag="pm", name="pwarm")
        for _ in range(n):
            nc.tensor.matmul(pw, wza, wzb, start=True, stop=True)

    def emit_merge_mlp(nt, warm=False):
        HT = NT // 2
        for half in range(2):
            ns = nt * NT + half * HT
            psm2 = pps.tile([128, 2, HT], F32, tag="ps", name="psm2")
            for ct in range(2):
                for h in range(H):
                    nc.tensor.matmul(psm2[:, ct, :], wmm(h, ct),
                                     msg_sb[:, h, ns:ns + HT],
                                     start=(h == 0), stop=(h == H - 1))
                nc.vector.tensor_copy(out=mm_sb[:, ct, ns:ns + HT],
                                      in_=psm2[:, ct, :])
            if warm:
                emit_warm_mms(3)
            phb = pps.tile([128, 2, HT], F32, tag="ps", name="phb")
            psh = phb[0:TD8, 0, :]
            nc.tensor.matmul(psh, wp1x(0), x_sb[:, 0, ns:ns + HT], start=True, stop=False)
            nc.tensor.matmul(psh, wp1x(1), x_sb[:, 1, ns:ns + HT], start=False, stop=False)
            nc.tensor.matmul(psh, wp1m(0), mm_sb[:, 0, ns:ns + HT], start=False, stop=False)
            nc.tensor.matmul(psh, wp1m(1), mm_sb[:, 1, ns:ns + HT], start=False, stop=True)
            nc.scalar.activation(out=h1[0:TD8, ns:ns + HT], in_=psh, func=AF.Relu,
                                 bias=be1_ap)
            if warm:
                emit_warm_mms(3)
            for ct in range(2):
                pso = phb[:, 1 - ct, :]
                nc.tensor.matmul(pso, wp2(ct), h1[:, ns:ns + HT], start=True, stop=True)
                nc.vector.tensor_copy(out=out_sb[:, ct, ns:ns + HT], in_=pso)
                nc.sync.dma_start(out=out_d[ct, :, ns:ns + HT],
                                  in_=out_sb[:, ct, ns:ns + HT])
            if warm:
                emit_warm_mms(3)

    seq = [(nt, h, bi) for nt in range(NTILES) for h in range(H)
           for bi in range(NSTEP)]
    pend = {}
    pm = None

    def emit_msg(idx):
        nonlocal pm
        nt, h, bi = seq[idx]
        if bi == 0:
            pm = ppm.tile([33, NT], F32, tag="pm", name="pm")
        e = pend.pop(idx)
        nc.tensor.matmul(pm, v1t[:, 2 * bi: 2 * bi + 2, 0:33], e,
                         start=(bi == 0), stop=(bi == NSTEP - 1),
                         perf_mode=mybir.MatmulPerfMode.DoubleRow)
        if bi == NSTEP - 1:
            emit_norm(pm, h, nt * NT)
            if h == H - 1:
                emit_merge_mlp(nt, warm=(nt == NTILES - 1))

    LOOK = 2
    for idx, step in enumerate(seq):
        pend[idx] = emit_scores(*step)
        if idx >= LOOK:
            emit_msg(idx - LOOK)
    for idx in range(len(seq) - LOOK, len(seq)):
        emit_msg(idx)


def build_program():
    nc = bacc.Bacc("TRN2", target_bir_lowering=False, debug=False)
    io = {}
    io["x_chunk"] = nc.dram_tensor("x_chunk", [2, 128, NCHUNK], F32R,
                                   kind="ExternalInput").ap()
    io["source_b"] = nc.dram_tensor("source_b", [2, 128, M], BF16,
                                    kind="ExternalInput").ap()
    io["cb"] = nc.dram_tensor("cb", [128, CB_END], BF16, kind="ExternalInput").ap()
    io["cf"] = nc.dram_tensor("cf", [128, CF_END], F32R, kind="ExternalInput").ap()
    io["out_chunk"] = nc.dram_tensor(
        "out_chunk", [2, 128, NCHUNK], F32, kind="ExternalOutput").ap()
    from contextlib import ExitStack
    with tile.TileContext(nc) as tc, ExitStack() as ctx:
        build_body(ctx, tc, io)
    nc.compile()
    return nc


def prep_weights(i):
    """Host-side folds; see module docstring."""
    import ml_dtypes
    bf = ml_dtypes.bfloat16
    f = np.float32
    a = {k: np.asarray(v, dtype=f) for k, v in i.items()}
    # head-contiguous channel permutation: c' = h*64+d  <- c = 4*d+h
    perm = (np.arange(H)[:, None] + H * np.arange(DIM)[None, :]).reshape(-1)

    def w1t(w):       # [D8, D] -> [128, 2*D8] (chunk-major)
        return np.ascontiguousarray(
            w.T.reshape(2, 128, D8).swapaxes(0, 1).reshape(128, 2 * D8))

    cbp = np.zeros((128, CB_END), np.float64)
    cfp = np.zeros((128, CF_END), np.float64)

    cbp[:, CB_WK:CB_WK + 64] = w1t(a["Wk1"])
    cbp[:, CB_WV:CB_WV + 64] = w1t(a["Wv1"])
    cfp[:, CF_WQ:CF_WQ + 64] = w1t(a["Wq1"])

    # cht: C_h = Ak'_h @ Aq'_h^T with first-proj biases folded into ones-rows
    wq2e = np.concatenate([a["Wq2"][perm].T, a["bq2"][perm][None, :]], 0)  # [33, 256]
    wk2e = np.concatenate([a["Wk2"][perm].T, a["bk2"][perm][None, :]], 0)
    for h in range(H):
        hs = slice(h * DIM, (h + 1) * DIM)
        Ak = wk2e[:, hs].astype(np.float64).copy()      # [33, 64]
        Aq = wq2e[:, hs].astype(np.float64).copy()
        Ak[32, :] += a["bk1"].astype(np.float64) @ Ak[0:32, :]
        Aq[32, :] += a["bq1"].astype(np.float64) @ Aq[0:32, :]
        C = Ak @ Aq.T                                   # [33(k-space), 33(q-space)]
        # cht[kq, h, p]: p<32 -> C[p, kq]; p=64 -> C[32, kq]
        blk = np.zeros((33, 128), np.float64)
        blk[:, 0:32] = C[0:32, :].T
        blk[:, 64] = C[32, :]
        cbp[0:33, CB_CHT + h * 128: CB_CHT + (h + 1) * 128] = blk

    # iv: [65, 34] selector: col 0 <- ones row (64), col 1+j <- v1 row (32+j)
    iv = np.zeros((65, 34), np.float64)
    iv[64, 0] = 1.0
    for j in range(32):
        iv[32 + j, 1 + j] = 1.0
    cbp[0:65, CB_IV:CB_IV + 34] = iv

    # wmm: mm = sum_h Wm2 @ [bias_h | W_h] @ msgv_h  (+ Wm2@bm1 + bm2 on h=0 col0)
    Wm2 = a["Wm2"].astype(np.float64)                   # [256, 32]
    for h in range(H):
        hs = perm[h * DIM:(h + 1) * DIM]
        Wm1p = a["Wm1"][:, hs].astype(np.float64)       # [32, 64]
        Wv2p = a["Wv2"][hs, :].astype(np.float64)       # [64, 32]
        bv2p = a["bv2"][hs].astype(np.float64)          # [64]
        W_h = Wm1p @ Wv2p                               # [32, 32]
        bias_h = Wm1p @ (bv2p + Wv2p @ a["bv1"].astype(np.float64))
        if h == 0:
            bias_h = bias_h + a["bm1"].astype(np.float64)
        Mfull = np.concatenate([bias_h[:, None], W_h], 1)   # [32, 33] (m1-space)
        M2 = Wm2 @ Mfull                                # [256, 33]
        if h == 0:
            M2[:, 0] += a["bm2"].astype(np.float64)
        for ct in range(2):
            cbp[0:33, CB_WMM + (h * 2 + ct) * 128: CB_WMM + (h * 2 + ct + 1) * 128] = \
                M2[ct * 128:(ct + 1) * 128, :].T

    # BN scale folds into Wp1; shift+bias into relu bias
    g1s = (a["g1"] / np.sqrt(np.float64(1.0) + np.float64(BN_EPS))).astype(np.float64)
    wp1x = a["Wp1"][:, 0:D].astype(np.float64) * g1s[:, None]     # [64, 256]
    wp1m = a["Wp1"][:, D:TD].astype(np.float64) * g1s[:, None]
    cfp[:, CF_WP1X:CF_WP1X + 128] = \
        wp1x.T.reshape(2, 128, TD8).swapaxes(0, 1).reshape(128, 128)
    cbp[:, CB_WP1M:CB_WP1M + 128] = \
        wp1m.T.reshape(2, 128, TD8).swapaxes(0, 1).reshape(128, 128)
    cfp[0:TD8, CF_BE1] = a["be1"].astype(np.float64) + g1s * a["bp1"].astype(np.float64)

    wp2t = np.concatenate(
        [a["Wp2"].T.reshape(TD8, 2, 128), a["bp2"].reshape(1, 2, 128)], 0)  # [65,2,128]
    cfp[0:TD8 + 1, CF_WP2:CF_WP2 + 256] = wp2t.reshape(TD8 + 1, 256)

    return {"cb": np.ascontiguousarray(cbp.astype(f).astype(bf)),
            "cf": np.ascontiguousarray(cfp.astype(f))}


_NC_CACHE = None


def _get_nc():
    global _NC_CACHE
    if _NC_CACHE is None:
        _NC_CACHE = build_program()
    return _NC_CACHE


def make_in_maps(inputs):
    import ml_dtypes
    w = prep_weights(inputs)
    x = np.ascontiguousarray(np.asarray(inputs["x"], np.float32))
    src = np.ascontiguousarray(np.asarray(inputs["source"], np.float32))
    in_maps = []
    for c in range(NCORES):
        b, ns = c // 4, (c % 4) * NCHUNK
        m = dict(w)
        m["x_chunk"] = np.ascontiguousarray(
            x[b].reshape(2, 128, N)[:, :, ns:ns + NCHUNK])
        m["source_b"] = np.ascontiguousarray(src[b].reshape(2, 128, M)).astype(
            ml_dtypes.bfloat16)
        in_maps.append(m)
    return in_maps


def assemble_out(results):
    out = np.empty((B, D, N), np.float32)
    for c in range(NCORES):
        b, ns = c // 4, (c % 4) * NCHUNK
        out[b].reshape(2, 128, N)[:, :, ns:ns + NCHUNK] = (
            results[c]["out_chunk"])
    return out


def kernel(**inputs):
    nc = _get_nc()
    res = bass_utils.run_bass_kernel_spmd(
        nc, make_in_maps(inputs), core_ids=list(range(NCORES)))
    return assemble_out(res.results)
